# revision 27
# baseline (speedup 1.0000x reference)
"""Trainium2 Bass kernel for nn_Attention_b (tanh-attention with masked_scatter).

Data-parallel over batch: each of 8 NeuronCores owns 4 batches. h_i and W
travel in bf16 (DVE gets its 2x 16-bit path only for bf16; PE bf16 == fp32r
speed), the score pipeline (m, u, y, sel) in fp16 for precision, exp weights
in bf16. End-to-end rel err ~9e-3 vs the 2e-2 gate. The whole per-core h_i
slice stays resident in SBUF; all DMAs are issued up front.

Schedule: phase 1 (score GEMM chunks) runs back-to-back on PE; a warm-up
AllGather brings up the CC rings during the first ~70us; grouped score
AllGathers fire as their chunks complete; each group's masked-scatter +
softmax + weighted-sum (phase 2/3) is interleaved into the instruction
streams at the point where its data is ready, so no engine queue parks
behind an unfinished dependency.
"""
import sys

for _p in ("/opt/trn_rl_repo",):
    if _p not in sys.path:
        sys.path.insert(0, _p)

import numpy as np

import concourse.bacc as bacc
import concourse.tile as tile
from concourse import mybir
from concourse.bass_utils import run_bass_kernel_spmd
from concourse.dve_ops import TENSOR_TENSOR_REDUCE
from concourse.masks import make_identity

NCORES = 8
B, S, H, A = 32, 2048, 1024, 256
BL = B // NCORES          # local batches per core
NEG = np.float32(-60000.0)   # fits fp16; exp(NEG-max) == 0 regardless

f32 = mybir.dt.float32
f16 = mybir.dt.float16
bf16 = mybir.dt.bfloat16

P1LIST = [128, 128, 256, 512, 512, 512]   # phase-1 (GEMM) chunking
CI = 512                                  # phase-2/3 chunking
NCH2 = S // CI
# AllGather groups as column ranges; group g covers phase-2 chunks P2G[g]
AGCOLS = [(0, 512), (512, 1536), (1536, 2048)]
P2G = [[0], [1, 2], [3]]
DMA_BCAST = False
KT_DVE = 3              # kt < KT_DVE -> fused TTR on DVE; rest DVE-mult+Act-reduce


def build_kernel():
    KT = H // 128             # contraction tiles
    AT = A // 128             # score tiles
    offs = np.concatenate([[0], np.cumsum(P1LIST)]).tolist()
    NCH1 = len(P1LIST)
    assert offs[-1] == S

    nc = bacc.Bacc("TRN2", target_bir_lowering=False, debug=False,
                   num_devices=NCORES)

    hi5 = nc.declare_dram_parameter("hi5", [128, KT * BL * S], bf16,
                                    isOutput=False)
    w1t = nc.declare_dram_parameter("w1t", [H, A], bf16, isOutput=False)
    cb2 = nc.declare_dram_parameter("cb2", [128, AT, BL], f32, isOutput=False)
    u2 = nc.declare_dram_parameter("u2", [128, AT], f16, isOutput=False)
    sel = nc.declare_dram_parameter("sel", [B + 1, BL, S], f16,
                                    isOutput=False)
    out = nc.declare_dram_parameter("out", [BL, H], f32, isOutput=True)

    with tile.TileContext(nc) as tc:
        with (
            tc.tile_pool(name="consts", bufs=1) as cp,
            tc.tile_pool(name="m", bufs=2) as mp,
            tc.tile_pool(name="tiny", bufs=2) as tp,
            tc.tile_pool(name="sely", bufs=2) as syp,
            tc.tile_pool(name="prod", bufs=4) as prp,
            tc.tile_pool(name="ebc", bufs=6) as ebp,
            tc.tile_pool(name="sacc", bufs=2) as sap,
            tc.tile_pool(name="pz", bufs=4, space="PSUM") as pz,
            tc.tile_pool(name="py", bufs=2, space="PSUM") as py,
            tc.tile_pool(name="pb", bufs=2, space="PSUM") as pb,
            tc.tile_pool(name="dram", bufs=3, space="DRAM") as dp,
        ):
            # ---- preload replicated constants
            w1_sb = cp.tile([128, KT, A], bf16)
            nc.sync.dma_start(
                out=w1_sb, in_=w1t.rearrange("(t p) a -> p t a", p=128))
            u_sb = cp.tile([128, AT], f16)
            nc.sync.dma_start(out=u_sb, in_=u2[:, :])
            cb_sb = cp.tile([128, AT, BL], f32)
            nc.sync.dma_start(out=cb_sb, in_=cb2[:, :, :])
            ident = cp.tile([128, 128], f32)
            make_identity(nc, ident)
            ones_sb = cp.tile([B + 1, 1], f16)
            nc.vector.memset(ones_sb, 1.0)
            wup = cp.tile([128, 512], bf16)
            nc.vector.memset(wup, 0.0)

            # ---- all input streams issued up front, h_i phase1-chunk-major
            hi_all = cp.tile([128, KT * BL * S], bf16)
            for i in range(NCH1):
                nc.sync.dma_start(
                    out=hi_all[:, KT * BL * offs[i] : KT * BL * offs[i + 1]],
                    in_=hi5[:, KT * BL * offs[i] : KT * BL * offs[i + 1]])
            sel_all = cp.tile([B + 1, BL, S], f16)
            nc.scalar.dma_start(out=sel_all, in_=sel[:, :, :])

            # PE p-state warm-up while DMAs stream
            wup_ps = pz.tile([128, 512], f32, tag="z")
            for i in range(10):
                nc.tensor.matmul(
                    wup_ps, wup[:, 0:128], wup[:, 0:512],
                    start=True, stop=True)

            def hi1(i):   # phase-1 chunk view [128, KT, BL, Ci]
                return hi_all[:, KT * BL * offs[i] : KT * BL * offs[i + 1]] \
                    .rearrange("p (t b s) -> p t b s", t=KT, b=BL)

            def hi2(j, kt, b):   # phase-2/3 slice [128, CI] of (kt, b)
                # column range [j*CI, (j+1)*CI) spans exact phase-1 chunks
                i0 = offs.index(j * CI)
                i1 = offs.index((j + 1) * CI)
                views = []
                for i in range(i0, i1):
                    Ci = P1LIST[i]
                    v = hi1(i)[:, kt, b, :]
                    views.append(v)
                return views   # list of [128, Ci] views covering CI cols

            # ---- per-chunk softmax stats (combined once at the end)
            mall = cp.tile([1, BL, NCH2], f32)
            lall = cp.tile([1, BL, NCH2], f32)
            saccs = [cp.tile([128, KT, BL, 3], f32, name=f"sacc{j}")
                     for j in range(NCH2)]
            y32s = [cp.tile([B + 1, CI], f16, name=f"y32_{j}")
                    for j in range(NCH2)]
            for j in range(NCH2):
                nc.gpsimd.memset(y32s[j][B : B + 1, :], 1.0)

            # per-AllGather-group DRAM score buffers
            ag_ins, ag_outs = [], []
            for g, (c0, c1) in enumerate(AGCOLS):
                wdt = c1 - c0
                agi = dp.tile([BL * wdt], f16, tag=f"agin{g}",
                              name=f"agin{g}")
                ago = dp.tile([B * wdt], f16, tag=f"agout{g}",
                              name=f"agout{g}", addr_space="Shared")
                ag_ins.append(agi)
                ag_outs.append(ago)

            def phase1(i, mid=None):
                Ci = P1LIST[i]
                hi_sb = hi1(i)
                g = next(gi for gi, (c0, c1) in enumerate(AGCOLS)
                         if c0 <= offs[i] < c1)
                c0, c1 = AGCOLS[g]
                m_r = mp.tile([128, AT, BL, Ci], f16, tag="m")
                for at in range(AT):
                    if at == 1 and mid is not None:
                        mid()
                    for b in range(BL):
                        z_ps = pz.tile([128, Ci], f32, tag="z")
                        for kt in range(KT):
                            nc.tensor.matmul(
                                z_ps,
                                w1_sb[:, kt, at * 128 : (at + 1) * 128],
                                hi_sb[:, kt, b, :],
                                start=(kt == 0), stop=(kt == KT - 1),
                            )
                        nc.scalar.activation(
                            out=m_r[:, at, b, :], in_=z_ps,
                            func=mybir.ActivationFunctionType.Tanh,
                            bias=cb_sb[:, at, b : b + 1], scale=1.0,
                        )
                y_sb = tp.tile([1, BL, Ci], f16, tag="ysb")
                for b in range(BL):
                    y_ps = py.tile([1, Ci], f32, tag="y")
                    for at in range(AT):
                        nc.tensor.matmul(
                            y_ps, u_sb[:, at : at + 1],
                            m_r[:, at, b : b + 1, :],
                            start=(at == 0), stop=(at == AT - 1),
                        )
                    nc.scalar.activation(
                        out=y_sb[:, b, :], in_=y_ps,
                        func=mybir.ActivationFunctionType.Copy)
                # stage this chunk's scores into its group's DRAM buffer
                loc = offs[i] - c0
                nc.scalar.dma_start(
                    out=ag_ins[g].rearrange("(o b s) -> o b s", o=1,
                                            s=c1 - c0)
                                  [:, :, loc : loc + Ci],
                    in_=y_sb)

            def ag_fire(g):
                c0, c1 = AGCOLS[g]
                nc.gpsimd.collective_compute(
                    "AllGather", mybir.AluOpType.bypass,
                    ins=[ag_ins[g][:]], outs=[ag_outs[g][:]],
                    replica_groups=[list(range(NCORES))],
                )
                # land gathered scores via the gpsimd queue (sits right
                # after this AG's completion on the same queue; sync is
                # still busy streaming h_i at this point)
                for j in P2G[g]:
                    loc = j * CI - c0
                    nc.gpsimd.dma_start(
                        out=y32s[j][:B, :],
                        in_=ag_outs[g].rearrange("(q s) -> q s", s=c1 - c0)
                                      [:, loc : loc + CI])

            def p23_selY(j):
                off = j * CI
                sel_c = sel_all[:, :, off : off + CI]
                y32 = y32s[j]
                selY = syp.tile([B + 1, BL, CI], f16, tag="selY",
                                name=f"selY{j}")
                nc.vector.tensor_mul(
                    selY, sel_c,
                    y32.rearrange("q (o s) -> q o s", o=1)
                       .broadcast_to([B + 1, BL, CI]))
                return selY

            def p23_btexp(j, selY):
                cmax = tp.tile([1, BL], f32, tag="cmax", name=f"cmax{j}",
                               bufs=3)
                nmnew = tp.tile([1, BL], f32, tag="nmnew", name=f"nm{j}",
                                bufs=3)
                ebcs = []
                for b in range(BL):
                    bt_ps = pb.tile([1, CI], f32, tag="bt")
                    nc.tensor.matmul(
                        bt_ps, ones_sb, selY[:, b : b + 1, :],
                        start=True, stop=True)
                    nc.vector.tensor_reduce(
                        out=cmax.rearrange("p (b o) -> p b o", o=1)
                                [:, b : b + 1],
                        in_=bt_ps,
                        axis=mybir.AxisListType.X, op=mybir.AluOpType.max)
                    nc.vector.tensor_scalar_mul(
                        nmnew[:, b : b + 1], cmax[:, b : b + 1], -1.0)
                    e4 = tp.tile([1, CI], bf16, tag="e4", bufs=6)
                    nc.scalar.activation(
                        out=e4, in_=bt_ps,
                        func=mybir.ActivationFunctionType.Exp,
                        bias=nmnew[:, b : b + 1], scale=1.0,
                        accum_out=lall[:, b, j : j + 1])
                    e_bc = ebp.tile([128, CI], bf16, tag="ebc")
                    nc.gpsimd.partition_broadcast(e_bc, e4)
                    ebcs.append(e_bc)
                nc.vector.tensor_copy(mall[:, :, j], cmax)
                return ebcs

            def p23_wsum_b(j, b, e_bc):
                sacc_j = saccs[j]
                ttr_scr = tp.tile([128, 1], bf16, tag="ttrscr")
                prs = tp.tile([128, 1], bf16, tag="prs")
                for kt in range(KT):
                    views = hi2(j, kt, b)
                    cw = 0
                    if kt < KT_DVE:
                        for vi, v in enumerate(views):
                            wv = v.shape[-1]
                            nc.vector._custom_dve(
                                TENSOR_TENSOR_REDUCE,
                                out=ttr_scr.broadcast_to([128, wv]),
                                in0=v,
                                in1=e_bc[:, cw : cw + wv],
                                s0=0.0, s1=1.0,
                                accum_out=sacc_j[:, kt, b, vi : vi + 1],
                            )
                            cw += wv
                    else:
                        prod = prp.tile([128, CI], bf16, tag="prod")
                        for v in views:
                            wv = v.shape[-1]
                            nc.vector.tensor_mul(
                                prod[:, cw : cw + wv], v,
                                e_bc[:, cw : cw + wv])
                            cw += wv
                        nc.scalar.activation(
                            out=prs.broadcast_to([128, CI]),
                            in_=prod,
                            func=mybir.ActivationFunctionType.Copy,
                            scale=1.0,
                            accum_out=sacc_j[:, kt, b, 0 : 1])

            # ---- schedule: phase1 back-to-back with phase2/3 interleaved
            # at the points where their collectives have completed
            phase1(0); phase1(1); phase1(2)
            ag_fire(0)                      # cols 0-512 (rides ring ~t70)
            phase1(3)
            # chunk 0 light chain overlaps the phase-1 tail: AG0 has
            # completed (~t71) by the time PE reaches this point (~t78)
            sY0 = p23_selY(0)
            eb0 = p23_btexp(0, sY0)
            phase1(4)
            ag_fire(1)                      # cols 512-1536
            hold = {}

            def mid5():
                sY1 = p23_selY(1)
                hold[1] = p23_btexp(1, sY1)

            phase1(5, mid=mid5)
            # threaded phase 2/3: chunk j+1's light chain is prepared in
            # the back half of chunk j's heavy stage
            ebs = {0: eb0, 1: hold[1]}
            sYs = {}
            for j in range(NCH2):
                if j not in ebs:
                    ebs[j] = p23_btexp(j, sYs[j])
                if j == 2:
                    ag_fire(2)              # cols 1536-2048
                for b in range(BL):
                    p23_wsum_b(j, b, ebs[j][b])
                    nx = j + 1
                    if nx >= 2 and nx < NCH2 and b == 2:
                        sYs[nx] = p23_selY(nx)

            # ---- finalize: combine chunk partials, divide, transpose, store
            M = tp.tile([1, BL], f32, tag="cmax", bufs=3)
            nc.vector.tensor_reduce(
                out=M.rearrange("p (b o) -> p b o", o=1), in_=mall,
                axis=mybir.AxisListType.X, op=mybir.AluOpType.max)
            nM = tp.tile([1, BL], f32, tag="nmnew", bufs=3)
            nc.vector.tensor_scalar_mul(nM, M, -1.0)
            w = tp.tile([1, BL, NCH2], f32, tag="w")
            for b in range(BL):
                nc.scalar.activation(
                    out=w[:, b, :], in_=mall[:, b, :],
                    func=mybir.ActivationFunctionType.Exp,
                    bias=nM[:, b : b + 1], scale=1.0)
            wl = tp.tile([1, BL, NCH2], f32, tag="wl")
            nc.vector.tensor_mul(wl, w, lall)
            lsum = tp.tile([1, BL], f32, tag="lsum")
            nc.vector.tensor_reduce(
                out=lsum.rearrange("p (b o) -> p b o", o=1), in_=wl,
                axis=mybir.AxisListType.X, op=mybir.AluOpType.add)
            il = tp.tile([1, BL], f32, tag="il")
            nc.vector.reciprocal(il, lsum)
            wn = tp.tile([1, BL, NCH2], f32, tag="wn")
            for b in range(BL):
                nc.vector.tensor_scalar_mul(wn[:, b, :], w[:, b, :],
                                            il[:, b : b + 1])
            wbc = ebp.tile([128, BL * NCH2], f32, tag="wbcf")
            nc.gpsimd.partition_broadcast(
                wbc, wn.rearrange("p b n -> p (b n)"))
            wbc_v = wbc.rearrange("p (b n) -> p b n", b=BL)
            sfin = sap.tile([128, KT, BL], f32, tag="sacc")
            nviews = [offs.index((j + 1) * CI) - offs.index(j * CI)
                      for j in range(NCH2)]
            for j in range(NCH2):
                for b in range(BL):
                    # fold the per-view sub-accumulators as we combine;
                    # vi>0 sub-slots only exist for the TTR kts (< KT_DVE)
                    for vi in range(nviews[j]):
                        ksl = slice(0, KT) if vi == 0 else slice(0, KT_DVE)
                        tmp = tp.tile([128, KT], f32, tag="ftmp")
                        nc.vector.tensor_scalar_mul(
                            tmp[:, ksl], saccs[j][:, ksl, b, vi],
                            wbc_v[:, b, j : j + 1])
                        if j == 0 and vi == 0:
                            nc.vector.tensor_copy(sfin[:, :, b], tmp)
                        else:
                            nc.vector.tensor_add(
                                sfin[:, ksl, b], sfin[:, ksl, b],
                                tmp[:, ksl])
            t_ps = py.tile([KT * BL, 128], f32, tag="y")
            nc.tensor.transpose(
                t_ps, sfin.rearrange("p t b -> p (t b)"), ident)
            t_sb = tp.tile([KT * BL, 128], f32, tag="tsb")
            nc.vector.tensor_copy(t_sb, t_ps)
            for t in range(KT):
                nc.sync.dma_start(
                    out=out[:, t * 128 : (t + 1) * 128],
                    in_=t_sb[t * BL : (t + 1) * BL, :])

    nc.compile()
    _split_pe_waits(nc)
    return nc


def _split_pe_waits(nc):
    """TRN2 PE instructions (S3_LW encoding) take a single sync-wait slot.
    Bacc's legalization misses some Matmults; hoist excess waits onto
    dedicated PE NoOps inserted directly before the offender."""
    for f in nc.m.functions:
        for bb in f.blocks:
            insts = bb.instructions
            i = 0
            while i < len(insts):
                ins = insts[i]
                if type(ins).__name__ in ("InstMatmult", "InstNoOp") and \
                        ins.engine == mybir.EngineType.PE:
                    si = ins.sync_info
                    if si is not None and len(si.on_wait) > 1:
                        extra, keep = si.on_wait[:-1], si.on_wait[-1:]
                        for w in extra:
                            nop = mybir.InstNoOp(
                                name=nc.get_next_instruction_name(),
                                ins=[], outs=[])
                            nop.engine = ins.engine
                            nop.sync_info = mybir.SyncInfo(
                                on_wait=[w], on_update=[])
                            nc.register_instruction(nop)
                            insts.insert(i, nop)
                            i += 1
                        si.on_wait = keep
                i += 1


def prep_inputs(h_i, h_t, mask, W, b, u):
    """Shard + lay out the full inputs for the 8 cores."""
    import ml_dtypes
    h_i = np.asarray(h_i, np.float32)
    h_t = np.asarray(h_t, np.float32)
    mask = np.asarray(mask, bool)
    W = np.asarray(W, np.float32)
    b = np.asarray(b, np.float32)
    u = np.asarray(u, np.float32)

    KT = H // 128
    AT = A // 128
    offs = np.concatenate([[0], np.cumsum(P1LIST)]).astype(int)
    w1t = np.ascontiguousarray(W[:, :H].T).astype(ml_dtypes.bfloat16)
    cb = h_t @ W[:, H:].T + b                                   # [B, A]
    cb2s = np.ascontiguousarray(
        cb.reshape(B, AT, 128).transpose(2, 1, 0))              # [128, AT, B]
    u2 = np.ascontiguousarray(
        u[:, 0].reshape(AT, 128).T).astype(np.float16)          # [128, AT]

    pos = np.clip(np.cumsum(mask.astype(np.int64), axis=0) - 1, 0, None)
    onehot = (np.arange(B)[None, :, None] == pos[:, None, :]) & mask[:, None, :]
    selall = onehot.astype(np.float16)                          # [B, B, S]
    negall = np.where(mask, np.float16(0), np.float16(NEG))     # [B, S]
    sel33 = np.concatenate([selall, negall[:, None, :]], axis=1)  # [B, B+1, S]

    h16 = h_i.astype(ml_dtypes.bfloat16)
    in_maps = []
    for c in range(NCORES):
        bs = slice(c * BL, (c + 1) * BL)
        # hi5[p, chunk ++ (t, b, s)] = h_i[b, off_i+s, t*128+p]
        hcf = h16[bs].reshape(BL, S, KT, 128)
        blocks = []
        for ci, off in zip(P1LIST, offs[:-1]):
            hc = hcf[:, off : off + ci]                     # [BL, ci, KT, 128]
            blocks.append(hc.transpose(3, 2, 0, 1).reshape(128, KT * BL * ci))
        hi5 = np.ascontiguousarray(np.concatenate(blocks, axis=1))
        in_maps.append({
            "hi5": hi5,
            "w1t": w1t,
            "cb2": np.ascontiguousarray(cb2s[:, :, bs]),
            "u2": u2,
            "sel": np.ascontiguousarray(sel33[bs].transpose(1, 0, 2)),
        })
    return in_maps


_NC_CACHE = {}


def _get_nc():
    if "nc" not in _NC_CACHE:
        _NC_CACHE["nc"] = build_kernel()
    return _NC_CACHE["nc"]


def kernel(h_i, h_t, mask, W, b, u):
    nc = _get_nc()
    in_maps = prep_inputs(h_i, h_t, mask, W, b, u)
    res = run_bass_kernel_spmd(nc, in_maps, list(range(NCORES)))
    return np.concatenate([res.results[c]["out"] for c in range(NCORES)],
                          axis=0)


# revision 28
# speedup vs baseline: 1.0373x; 1.0373x over previous
"""Trainium2 Bass kernel for nn_Attention_b (tanh-attention with masked_scatter).

Data-parallel over batch: each of 8 NeuronCores owns 4 batches. h_i and W
travel in bf16 (DVE gets its 2x 16-bit path only for bf16; PE bf16 == fp32r
speed), the score pipeline (m, u, y, sel) in fp16 for precision, exp weights
in bf16. End-to-end rel err ~9e-3 vs the 2e-2 gate. The whole per-core h_i
slice stays resident in SBUF; all DMAs are issued up front.

Schedule: phase 1 (score GEMM chunks) runs back-to-back on PE; a warm-up
AllGather brings up the CC rings during the first ~70us; grouped score
AllGathers fire as their chunks complete; each group's masked-scatter +
softmax + weighted-sum (phase 2/3) is interleaved into the instruction
streams at the point where its data is ready, so no engine queue parks
behind an unfinished dependency.
"""
import sys

for _p in ("/opt/trn_rl_repo",):
    if _p not in sys.path:
        sys.path.insert(0, _p)

import numpy as np

import concourse.bacc as bacc
import concourse.tile as tile
from concourse import mybir
from concourse.bass_utils import run_bass_kernel_spmd
from concourse.dve_ops import TENSOR_TENSOR_REDUCE
from concourse.masks import make_identity

NCORES = 8
B, S, H, A = 32, 2048, 1024, 256
BL = B // NCORES          # local batches per core
NEG = np.float32(-60000.0)   # fits fp16; exp(NEG-max) == 0 regardless

f32 = mybir.dt.float32
f16 = mybir.dt.float16
bf16 = mybir.dt.bfloat16

P1LIST = [128, 128, 256, 512, 512, 512]   # phase-1 (GEMM) chunking
CI = 512                                  # phase-2/3 chunking
NCH2 = S // CI
# AllGather groups as column ranges; group g covers phase-2 chunks P2G[g]
AGCOLS = [(0, 512), (512, 1536), (1536, 2048)]
P2G = [[0], [1, 2], [3]]
DMA_BCAST = False
KT_DVE = 3              # kt < KT_DVE -> fused TTR on DVE; rest DVE-mult+Act-reduce


def build_kernel():
    KT = H // 128             # contraction tiles
    AT = A // 128             # score tiles
    offs = np.concatenate([[0], np.cumsum(P1LIST)]).tolist()
    NCH1 = len(P1LIST)
    assert offs[-1] == S

    nc = bacc.Bacc("TRN2", target_bir_lowering=False, debug=False,
                   num_devices=NCORES)

    hi5 = nc.declare_dram_parameter("hi5", [128, KT * BL * S], bf16,
                                    isOutput=False)
    w1t = nc.declare_dram_parameter("w1t", [H, A], bf16, isOutput=False)
    cb2 = nc.declare_dram_parameter("cb2", [128, AT, BL], f32, isOutput=False)
    u2 = nc.declare_dram_parameter("u2", [128, AT], f16, isOutput=False)
    sel = nc.declare_dram_parameter("sel", [B + 1, BL, S], f16,
                                    isOutput=False)
    out = nc.declare_dram_parameter("out", [BL, H], f32, isOutput=True)

    with tile.TileContext(nc) as tc:
        with (
            tc.tile_pool(name="consts", bufs=1) as cp,
            tc.tile_pool(name="m", bufs=2) as mp,
            tc.tile_pool(name="tiny", bufs=2) as tp,
            tc.tile_pool(name="sely", bufs=2) as syp,
            tc.tile_pool(name="prod", bufs=4) as prp,
            tc.tile_pool(name="ebc", bufs=6) as ebp,
            tc.tile_pool(name="sacc", bufs=2) as sap,
            tc.tile_pool(name="pz", bufs=2, space="PSUM") as pz,
            tc.tile_pool(name="py", bufs=2, space="PSUM") as py,
            tc.tile_pool(name="pb", bufs=4, space="PSUM") as pb,
            tc.tile_pool(name="dram", bufs=3, space="DRAM") as dp,
        ):
            # ---- preload replicated constants
            w1_sb = cp.tile([128, KT, A], bf16)
            nc.sync.dma_start(
                out=w1_sb, in_=w1t.rearrange("(t p) a -> p t a", p=128))
            u_sb = cp.tile([128, AT], f16)
            nc.sync.dma_start(out=u_sb, in_=u2[:, :])
            cb_sb = cp.tile([128, AT, BL], f32)
            nc.sync.dma_start(out=cb_sb, in_=cb2[:, :, :])
            ident = cp.tile([128, 128], f32)
            make_identity(nc, ident)
            ones_sb = cp.tile([B + 1, 1], f16)
            nc.vector.memset(ones_sb, 1.0)
            wup = cp.tile([128, 512], bf16)
            nc.vector.memset(wup, 0.0)

            # ---- all input streams issued up front, h_i phase1-chunk-major
            hi_all = cp.tile([128, KT * BL * S], bf16)
            for i in range(NCH1):
                nc.sync.dma_start(
                    out=hi_all[:, KT * BL * offs[i] : KT * BL * offs[i + 1]],
                    in_=hi5[:, KT * BL * offs[i] : KT * BL * offs[i + 1]])
            sel_all = cp.tile([B + 1, BL, S], f16)
            nc.scalar.dma_start(out=sel_all, in_=sel[:, :, :])

            # PE p-state warm-up while DMAs stream
            wup_ps = pz.tile([128, 512], f32, tag="z")
            for i in range(10):
                nc.tensor.matmul(
                    wup_ps, wup[:, 0:128], wup[:, 0:512],
                    start=True, stop=True)

            def hi1(i):   # phase-1 chunk view [128, KT, BL, Ci]
                return hi_all[:, KT * BL * offs[i] : KT * BL * offs[i + 1]] \
                    .rearrange("p (t b s) -> p t b s", t=KT, b=BL)

            def hi2(j, kt, b):   # phase-2/3 slice [128, CI] of (kt, b)
                # column range [j*CI, (j+1)*CI) spans exact phase-1 chunks
                i0 = offs.index(j * CI)
                i1 = offs.index((j + 1) * CI)
                views = []
                for i in range(i0, i1):
                    Ci = P1LIST[i]
                    v = hi1(i)[:, kt, b, :]
                    views.append(v)
                return views   # list of [128, Ci] views covering CI cols

            # ---- per-chunk softmax stats (combined once at the end)
            mall = cp.tile([1, BL, NCH2], f32)
            lall = cp.tile([1, BL, NCH2], f32)
            saccs = [cp.tile([128, KT, BL, 3], f32, name=f"sacc{j}")
                     for j in range(NCH2)]
            y32s = [cp.tile([B + 1, CI], f16, name=f"y32_{j}")
                    for j in range(NCH2)]
            for j in range(NCH2):
                nc.gpsimd.memset(y32s[j][B : B + 1, :], 1.0)

            # per-AllGather-group DRAM score buffers
            ag_ins, ag_outs = [], []
            for g, (c0, c1) in enumerate(AGCOLS):
                wdt = c1 - c0
                agi = dp.tile([BL * wdt], f16, tag=f"agin{g}",
                              name=f"agin{g}")
                ago = dp.tile([B * wdt], f16, tag=f"agout{g}",
                              name=f"agout{g}", addr_space="Shared")
                ag_ins.append(agi)
                ag_outs.append(ago)

            def phase1(i, mid=None):
                Ci = P1LIST[i]
                hi_sb = hi1(i)
                g = next(gi for gi, (c0, c1) in enumerate(AGCOLS)
                         if c0 <= offs[i] < c1)
                c0, c1 = AGCOLS[g]
                m_r = mp.tile([128, AT, BL, Ci], f16, tag="m")
                for at in range(AT):
                    if at == 1 and mid is not None:
                        mid()
                    for b in range(BL):
                        z_ps = pz.tile([128, Ci], f32, tag="z")
                        for kt in range(KT):
                            nc.tensor.matmul(
                                z_ps,
                                w1_sb[:, kt, at * 128 : (at + 1) * 128],
                                hi_sb[:, kt, b, :],
                                start=(kt == 0), stop=(kt == KT - 1),
                            )
                        nc.scalar.activation(
                            out=m_r[:, at, b, :], in_=z_ps,
                            func=mybir.ActivationFunctionType.Tanh,
                            bias=cb_sb[:, at, b : b + 1], scale=1.0,
                        )
                y_sb = tp.tile([1, BL, Ci], f16, tag="ysb")
                for b in range(BL):
                    y_ps = py.tile([1, Ci], f32, tag="y")
                    for at in range(AT):
                        nc.tensor.matmul(
                            y_ps, u_sb[:, at : at + 1],
                            m_r[:, at, b : b + 1, :],
                            start=(at == 0), stop=(at == AT - 1),
                        )
                    nc.scalar.activation(
                        out=y_sb[:, b, :], in_=y_ps,
                        func=mybir.ActivationFunctionType.Copy)
                # stage this chunk's scores into its group's DRAM buffer
                loc = offs[i] - c0
                nc.scalar.dma_start(
                    out=ag_ins[g].rearrange("(o b s) -> o b s", o=1,
                                            s=c1 - c0)
                                  [:, :, loc : loc + Ci],
                    in_=y_sb)

            def ag_fire(g):
                c0, c1 = AGCOLS[g]
                nc.gpsimd.collective_compute(
                    "AllGather", mybir.AluOpType.bypass,
                    ins=[ag_ins[g][:]], outs=[ag_outs[g][:]],
                    replica_groups=[list(range(NCORES))],
                )
                # land gathered scores via the gpsimd queue (sits right
                # after this AG's completion on the same queue; sync is
                # still busy streaming h_i at this point)
                for j in P2G[g]:
                    loc = j * CI - c0
                    nc.gpsimd.dma_start(
                        out=y32s[j][:B, :],
                        in_=ag_outs[g].rearrange("(q s) -> q s", s=c1 - c0)
                                      [:, loc : loc + CI])

            def p23_selY(j):
                off = j * CI
                sel_c = sel_all[:, :, off : off + CI]
                y32 = y32s[j]
                selY = syp.tile([B + 1, BL, CI], f16, tag="selY",
                                name=f"selY{j}")
                nc.vector.tensor_mul(
                    selY, sel_c,
                    y32.rearrange("q (o s) -> q o s", o=1)
                       .broadcast_to([B + 1, BL, CI]))
                return selY

            def p23_btexp(j, selY):
                cmax = tp.tile([1, BL], f32, tag="cmax", name=f"cmax{j}",
                               bufs=3)
                nmnew = tp.tile([1, BL], f32, tag="nmnew", name=f"nm{j}",
                                bufs=3)
                ebcs = []
                for b in range(BL):
                    bt_ps = pb.tile([1, CI], f32, tag="bt")
                    nc.tensor.matmul(
                        bt_ps, ones_sb, selY[:, b : b + 1, :],
                        start=True, stop=True)
                    nc.vector.tensor_reduce(
                        out=cmax.rearrange("p (b o) -> p b o", o=1)
                                [:, b : b + 1],
                        in_=bt_ps,
                        axis=mybir.AxisListType.X, op=mybir.AluOpType.max)
                    nc.vector.tensor_scalar_mul(
                        nmnew[:, b : b + 1], cmax[:, b : b + 1], -1.0)
                    e4 = tp.tile([1, CI], bf16, tag="e4", bufs=6)
                    nc.scalar.activation(
                        out=e4, in_=bt_ps,
                        func=mybir.ActivationFunctionType.Exp,
                        bias=nmnew[:, b : b + 1], scale=1.0,
                        accum_out=lall[:, b, j : j + 1])
                    e_bc = ebp.tile([128, CI], bf16, tag="ebc")
                    nc.gpsimd.partition_broadcast(e_bc, e4)
                    ebcs.append(e_bc)
                nc.vector.tensor_copy(mall[:, :, j], cmax)
                return ebcs

            def p23_wsum_b(j, b, e_bc):
                sacc_j = saccs[j]
                ttr_scr = tp.tile([128, 1], bf16, tag="ttrscr")
                prs = tp.tile([128, 1], bf16, tag="prs")
                for kt in range(KT):
                    views = hi2(j, kt, b)
                    cw = 0
                    if kt < KT_DVE:
                        for vi, v in enumerate(views):
                            wv = v.shape[-1]
                            nc.vector._custom_dve(
                                TENSOR_TENSOR_REDUCE,
                                out=ttr_scr.broadcast_to([128, wv]),
                                in0=v,
                                in1=e_bc[:, cw : cw + wv],
                                s0=0.0, s1=1.0,
                                accum_out=sacc_j[:, kt, b, vi : vi + 1],
                            )
                            cw += wv
                    else:
                        prod = prp.tile([128, CI], bf16, tag="prod")
                        for v in views:
                            wv = v.shape[-1]
                            nc.vector.tensor_mul(
                                prod[:, cw : cw + wv], v,
                                e_bc[:, cw : cw + wv])
                            cw += wv
                        nc.scalar.activation(
                            out=prs.broadcast_to([128, CI]),
                            in_=prod,
                            func=mybir.ActivationFunctionType.Copy,
                            scale=1.0,
                            accum_out=sacc_j[:, kt, b, 0 : 1])

            # ---- schedule: phase1 back-to-back with phase2/3 interleaved
            # at the points where their collectives have completed
            phase1(0); phase1(1); phase1(2)
            ag_fire(0)                      # cols 0-512 (rides ring ~t70)
            phase1(3)
            # chunk 0 light chain overlaps the phase-1 tail: AG0 has
            # completed (~t71) by the time PE reaches this point (~t78)
            sY0 = p23_selY(0)
            eb0 = p23_btexp(0, sY0)
            phase1(4)
            ag_fire(1)                      # cols 512-1536
            hold = {}

            def mid5():
                sY1 = p23_selY(1)
                hold[1] = p23_btexp(1, sY1)

            phase1(5, mid=mid5)
            # threaded phase 2/3: chunk j+1's light chain is prepared in
            # the back half of chunk j's heavy stage
            ebs = {0: eb0, 1: hold[1]}
            sYs = {}
            for j in range(NCH2):
                if j not in ebs:
                    ebs[j] = p23_btexp(j, sYs[j])
                if j == 2:
                    ag_fire(2)              # cols 1536-2048
                for b in range(BL):
                    p23_wsum_b(j, b, ebs[j][b])
                    nx = j + 1
                    if nx >= 2 and nx < NCH2 and b == 2:
                        sYs[nx] = p23_selY(nx)

            # ---- finalize: combine chunk partials, divide, transpose, store
            M = tp.tile([1, BL], f32, tag="cmax", bufs=3)
            nc.vector.tensor_reduce(
                out=M.rearrange("p (b o) -> p b o", o=1), in_=mall,
                axis=mybir.AxisListType.X, op=mybir.AluOpType.max)
            nM = tp.tile([1, BL], f32, tag="nmnew", bufs=3)
            nc.vector.tensor_scalar_mul(nM, M, -1.0)
            w = tp.tile([1, BL, NCH2], f32, tag="w")
            for b in range(BL):
                nc.scalar.activation(
                    out=w[:, b, :], in_=mall[:, b, :],
                    func=mybir.ActivationFunctionType.Exp,
                    bias=nM[:, b : b + 1], scale=1.0)
            wl = tp.tile([1, BL, NCH2], f32, tag="wl")
            nc.vector.tensor_mul(wl, w, lall)
            lsum = tp.tile([1, BL], f32, tag="lsum")
            nc.vector.tensor_reduce(
                out=lsum.rearrange("p (b o) -> p b o", o=1), in_=wl,
                axis=mybir.AxisListType.X, op=mybir.AluOpType.add)
            il = tp.tile([1, BL], f32, tag="il")
            nc.vector.reciprocal(il, lsum)
            wn = tp.tile([1, BL, NCH2], f32, tag="wn")
            for b in range(BL):
                nc.vector.tensor_scalar_mul(wn[:, b, :], w[:, b, :],
                                            il[:, b : b + 1])
            wbc = ebp.tile([128, BL * NCH2], f32, tag="wbcf")
            nc.gpsimd.partition_broadcast(
                wbc, wn.rearrange("p b n -> p (b n)"))
            wbc_v = wbc.rearrange("p (b n) -> p b n", b=BL)
            sfin = sap.tile([128, KT, BL], f32, tag="sacc")
            nviews = [offs.index((j + 1) * CI) - offs.index(j * CI)
                      for j in range(NCH2)]
            for j in range(NCH2):
                for b in range(BL):
                    # fold the per-view sub-accumulators as we combine;
                    # vi>0 sub-slots only exist for the TTR kts (< KT_DVE)
                    for vi in range(nviews[j]):
                        ksl = slice(0, KT) if vi == 0 else slice(0, KT_DVE)
                        tmp = tp.tile([128, KT], f32, tag="ftmp")
                        nc.vector.tensor_scalar_mul(
                            tmp[:, ksl], saccs[j][:, ksl, b, vi],
                            wbc_v[:, b, j : j + 1])
                        if j == 0 and vi == 0:
                            nc.vector.tensor_copy(sfin[:, :, b], tmp)
                        else:
                            nc.vector.tensor_add(
                                sfin[:, ksl, b], sfin[:, ksl, b],
                                tmp[:, ksl])
            t_ps = py.tile([KT * BL, 128], f32, tag="y")
            nc.tensor.transpose(
                t_ps, sfin.rearrange("p t b -> p (t b)"), ident)
            t_sb = tp.tile([KT * BL, 128], f32, tag="tsb")
            nc.vector.tensor_copy(t_sb, t_ps)
            for t in range(KT):
                nc.sync.dma_start(
                    out=out[:, t * 128 : (t + 1) * 128],
                    in_=t_sb[t * BL : (t + 1) * BL, :])

    nc.compile()
    _split_pe_waits(nc)
    return nc


def _split_pe_waits(nc):
    """TRN2 PE instructions (S3_LW encoding) take a single sync-wait slot.
    Bacc's legalization misses some Matmults; hoist excess waits onto
    dedicated PE NoOps inserted directly before the offender."""
    for f in nc.m.functions:
        for bb in f.blocks:
            insts = bb.instructions
            i = 0
            while i < len(insts):
                ins = insts[i]
                if type(ins).__name__ in ("InstMatmult", "InstNoOp") and \
                        ins.engine == mybir.EngineType.PE:
                    si = ins.sync_info
                    if si is not None and len(si.on_wait) > 1:
                        extra, keep = si.on_wait[:-1], si.on_wait[-1:]
                        for w in extra:
                            nop = mybir.InstNoOp(
                                name=nc.get_next_instruction_name(),
                                ins=[], outs=[])
                            nop.engine = ins.engine
                            nop.sync_info = mybir.SyncInfo(
                                on_wait=[w], on_update=[])
                            nc.register_instruction(nop)
                            insts.insert(i, nop)
                            i += 1
                        si.on_wait = keep
                i += 1


def prep_inputs(h_i, h_t, mask, W, b, u):
    """Shard + lay out the full inputs for the 8 cores."""
    import ml_dtypes
    h_i = np.asarray(h_i, np.float32)
    h_t = np.asarray(h_t, np.float32)
    mask = np.asarray(mask, bool)
    W = np.asarray(W, np.float32)
    b = np.asarray(b, np.float32)
    u = np.asarray(u, np.float32)

    KT = H // 128
    AT = A // 128
    offs = np.concatenate([[0], np.cumsum(P1LIST)]).astype(int)
    w1t = np.ascontiguousarray(W[:, :H].T).astype(ml_dtypes.bfloat16)
    cb = h_t @ W[:, H:].T + b                                   # [B, A]
    cb2s = np.ascontiguousarray(
        cb.reshape(B, AT, 128).transpose(2, 1, 0))              # [128, AT, B]
    u2 = np.ascontiguousarray(
        u[:, 0].reshape(AT, 128).T).astype(np.float16)          # [128, AT]

    pos = np.clip(np.cumsum(mask.astype(np.int64), axis=0) - 1, 0, None)
    onehot = (np.arange(B)[None, :, None] == pos[:, None, :]) & mask[:, None, :]
    selall = onehot.astype(np.float16)                          # [B, B, S]
    negall = np.where(mask, np.float16(0), np.float16(NEG))     # [B, S]
    sel33 = np.concatenate([selall, negall[:, None, :]], axis=1)  # [B, B+1, S]

    h16 = h_i.astype(ml_dtypes.bfloat16)
    in_maps = []
    for c in range(NCORES):
        bs = slice(c * BL, (c + 1) * BL)
        # hi5[p, chunk ++ (t, b, s)] = h_i[b, off_i+s, t*128+p]
        hcf = h16[bs].reshape(BL, S, KT, 128)
        blocks = []
        for ci, off in zip(P1LIST, offs[:-1]):
            hc = hcf[:, off : off + ci]                     # [BL, ci, KT, 128]
            blocks.append(hc.transpose(3, 2, 0, 1).reshape(128, KT * BL * ci))
        hi5 = np.ascontiguousarray(np.concatenate(blocks, axis=1))
        in_maps.append({
            "hi5": hi5,
            "w1t": w1t,
            "cb2": np.ascontiguousarray(cb2s[:, :, bs]),
            "u2": u2,
            "sel": np.ascontiguousarray(sel33[bs].transpose(1, 0, 2)),
        })
    return in_maps


_NC_CACHE = {}


def _get_nc():
    if "nc" not in _NC_CACHE:
        _NC_CACHE["nc"] = build_kernel()
    return _NC_CACHE["nc"]


def kernel(h_i, h_t, mask, W, b, u):
    nc = _get_nc()
    in_maps = prep_inputs(h_i, h_t, mask, W, b, u)
    res = run_bass_kernel_spmd(nc, in_maps, list(range(NCORES)))
    return np.concatenate([res.results[c]["out"] for c in range(NCORES)],
                          axis=0)
